# revision 8
# baseline (speedup 1.0000x reference)
"""Trainium2 Bass kernel for CustomApplyTimeChannel.

Per (batch b, block n): y[b,n,:] = full_conv(x[b,n,:1096], h[b,n,:24]),
then overlap-add with hop T=1096 into out[b, :15367].
Pure data parallel over batch across 8 NeuronCores (16 b/core); per-core
rows p = b*14 + n (b-major), split into two 112-row partition tiles at a
batch boundary.  b-major descriptor order walks DRAM contiguously, which
measures ~10x faster per body on real HW than the n-major order (the DMA
descriptor path, not compute, dominates real execution time here).

Engine split of the 24 taps (per tile):
  - taps 1..18 / 1..17 on PE as bf16 diag-weight matmuls accumulating
    fp32 in PSUM.  The ISA caps a matmul at one PSUM bank (512 fp32
    cols), so the 1119 output columns are split into three pieces with
    SEPARATE PSUM tiles and SEPARATE bf16 x-window tiles per piece —
    Tile's dependency tracking is tile-coarse, so per-piece tiles let
    each piece's fold run as soon as that piece's group retires.
  - tap 0 + the remaining taps on DVE as fp32 TensorScalarPtr MACs.
  - Constants are built on-device: tile-0 diag weights + the two
    overlap-add shift matrices on Pool via affine_select (from a
    stride-0 broadcast of h), tile-1 diag weights on ACT as eye*h.
  - ACT casts x to bf16 (three window tiles per partition tile).
The overlap-add shifts frame tails down one partition with a holed
shift-matrix matmul per tile into PSUM plus two narrow DVE adds (the
tile split is at a batch boundary, so no tail crosses tiles).  All DMAs ride
the SP/ACT hardware DGE queues (gpsimd software DGE is far slower per
descriptor).

This walrus accepts only ONE sync wait per instruction; any multi-wait
instruction is legalized post-schedule by hoisting extra waits onto
single-wait Drain instructions on the same engine (see
_legalize_single_wait).

The host path compiles the PJRT executable once and caches it; repeat
kernel() calls only device_put the inputs and dispatch.  If BASS_TRACE
is set, execution routes through run_bass_kernel_spmd so NTFF profiling
hooks (where available) still observe the kernel.
"""

import os
import sys

sys.path.insert(0, "/opt/trn_rl_repo")

import numpy as np

from concourse import bass, tile, mybir

# Problem constants (hardcoded; kernel.py must be self-contained).
B = 128          # total batch
NB = 14          # channel blocks
T = 1096         # time samples per block
L = 24           # taps
F = T + L - 1    # frame length 1119
OUT_LEN = (NB - 1) * T + F   # 15367
N_CORES = 8
BC = B // N_CORES            # 16 batches per core
ROWS = NB * BC               # 224 rows per core
P0 = 128                     # constant-tile partition width
PR = 112                     # rows per partition tile (8 batches x 14 blocks)

FP32 = mybir.dt.float32
BF16 = mybir.dt.bfloat16

# per-tile tap assignment (tap 0 initializes Y on DVE)
PE_TAPS = [list(range(1, 19)), list(range(1, 18))]
DVE_TAPS = [list(range(19, 24)), list(range(18, 24))]

# rows are b-major (p = b*14 + n): DMA descriptors then walk DRAM
# contiguously, which measures ~2x faster per body on real HW than the
# n-major order; the two 112-row tiles split at a batch boundary, so no
# frame tail crosses tiles (n = 0 rows start each batch).

# column pieces of the 1119-wide output, one PSUM bank each
PIECES = [(0, 512), (512, 1024), (1024, F)]

_CACHE = {}


def _legalize_single_wait(nc):
    """The walrus build here allows ONE sync wait per instruction.  For any
    instruction carrying N>1 waits, hoist N-1 of them onto bare Drain
    instructions on the SAME engine placed immediately before it: the engine
    sequencer executes in order, so the conjunction of waits is preserved.
    Must run after Tile's scheduler assigned sync_info and after the Tile
    instruction hook was popped (i.e. from within _drain_and_barrier)."""
    for f in nc.m.functions:
        for blk in f.blocks:
            snapshot = list(blk.instructions)
            if not any(
                i.sync_info is not None and len(i.sync_info.on_wait) > 1
                for i in snapshot
            ):
                continue
            created = set()
            new_list = []
            for ins in snapshot:
                si = ins.sync_info
                if si is not None and len(si.on_wait) > 1:
                    waits = list(si.on_wait)
                    for w in waits[:-1]:
                        d = nc.sync.drain()
                        d.ins.engine = ins.engine
                        d.ins.sync_info = mybir.SyncInfo(on_wait=[w], on_update=[])
                        created.add(id(d.ins))
                        new_list.append(d.ins)
                    ins.sync_info = mybir.SyncInfo(
                        on_wait=[waits[-1]], on_update=list(si.on_update)
                    )
                new_list.append(ins)
            # nc.sync.drain() appended the new drains at the end of the
            # current block; drop those trailing copies everywhere and
            # install the ordered list.
            for f2 in nc.m.functions:
                for blk2 in f2.blocks:
                    if blk2 is blk:
                        blk2.instructions[:] = new_list
                    else:
                        blk2.instructions[:] = [
                            i for i in blk2.instructions if id(i) not in created
                        ]


def _patch_drain_split():
    """Tile's kernel-tail drain carries one wait per outstanding processor;
    split it into a chain of single-wait drains, and legalize any other
    multi-wait instruction the same way."""
    if getattr(tile.TileContext, "_drain_split_patched", False):
        return
    from concourse.vector_clock import ScopedClock

    def _drain_and_barrier(self, tick_clock, wait_clock):
        _legalize_single_wait(self.nc)
        drain_inst = self.nc.sync.drain()
        wait_clock.add_sem_waits(
            drain_inst.ins, ScopedClock({None: tick_clock.global_clock})
        )
        si = drain_inst.ins.sync_info
        if si is not None and len(si.on_wait) > 1:
            waits = list(si.on_wait)
            drain_inst.ins.sync_info = mybir.SyncInfo(
                on_wait=[waits[0]], on_update=list(si.on_update)
            )
            for w in waits[1:]:
                d2 = self.nc.sync.drain()
                d2.ins.sync_info = mybir.SyncInfo(on_wait=[w], on_update=[])
        self.nc.all_engine_barrier()
        popped = self.nc._tile_sem_poison_stack.pop()
        assert popped is self._sem_poison
        self.nc.clear_and_free_semaphores(list(self.sems.allocated().values()))
        self.nc.all_engine_barrier()

    tile.TileContext._drain_and_barrier = _drain_and_barrier
    tile.TileContext._drain_split_patched = True


def _audit_single_wait(nc):
    bad = []
    for f in nc.m.functions:
        for blk in f.blocks:
            for ins in blk.instructions:
                si = ins.sync_info
                if si is not None and len(si.on_wait) > 1:
                    bad.append((type(ins).__name__, ins.name, len(si.on_wait)))
    if bad:
        raise RuntimeError(f"instructions with >1 sync wait: {bad}")


def _build_nc():
    _patch_drain_split()
    nc = bass.Bass()
    x_ext = nc.declare_dram_parameter("x", [BC, NB, T], FP32, isOutput=False)
    h_ext = nc.declare_dram_parameter("h", [BC, NB, L], FP32, isOutput=False)
    out_ext = nc.declare_dram_parameter("out", [BC, OUT_LEN], FP32, isOutput=True)

    ov = out_ext[:, : NB * T].rearrange("b (n t) -> b n t", n=NB, t=T)

    with tile.TileContext(nc) as tc:
        with (
            tc.tile_pool(name="main", bufs=1) as pool,
            tc.tile_pool(name="ps", bufs=1, space="PSUM") as pspool,
        ):
            ONES = pool.tile([P0, P0], FP32, tag="ones")
            S16 = pool.tile([P0, P0], FP32, tag="s16")
            EY = pool.tile([P0, P0], FP32, tag="ey")
            X0 = pool.tile([PR, T], FP32, tag="x0")
            X1 = pool.tile([PR, T], FP32, tag="x1")
            H0 = pool.tile([PR, L], FP32, tag="h0")
            H1 = pool.tile([PR, L], FP32, tag="h1")
            Y0 = pool.tile([PR, F], FP32, tag="y0")
            Y1 = pool.tile([PR, F], FP32, tag="y1")
            X, H, Y, PN = [X0, X1], [H0, H1], [Y0, Y1], [PR, PR]
            # per-piece bf16 x windows: piece p of tap j reads columns
            # [23-j+c0, 23-j+c1) of the padded x; with per-piece tiles the
            # local window is [23-j, 23-j+piece_w) in every piece.
            XA = [
                pool.tile([PN[k], 535], BF16, tag=f"xa{k}", name=f"xa{k}")
                for k in range(2)
            ]
            XBt = [
                pool.tile([PN[k], 535], BF16, tag=f"xb{k}", name=f"xb{k}")
                for k in range(2)
            ]
            XC = [
                pool.tile([PN[k], 118], BF16, tag=f"xc{k}", name=f"xc{k}")
                for k in range(2)
            ]
            # per-piece PSUM accumulators + tail-shift tiles: 8 banks exactly
            PA = [
                pspool.tile([PN[k], 512], FP32, tag=f"pa{k}", name=f"pa{k}")
                for k in range(2)
            ]
            PB = [
                pspool.tile([PN[k], 512], FP32, tag=f"pb{k}", name=f"pb{k}")
                for k in range(2)
            ]
            PC = [
                pspool.tile([PN[k], F - 1024], FP32, tag=f"pc{k}", name=f"pc{k}")
                for k in range(2)
            ]
            PP = [PA, PB, PC]
            TP0 = pspool.tile([PR, L - 1], FP32, tag="tp0")
            TP1 = pspool.tile([PR, L - 1], FP32, tag="tp1")
            TP = [TP0, TP1]

            # loads: x on SP, h on ACT (both hardware DGE), b-major
            NS = [(0, 8), (8, BC)]
            for k, (b0, b1) in enumerate(NS):
                nc.sync.dma_start(out=X[k][:], in_=x_ext[b0:b1])
                nc.scalar.dma_start(out=H[k][:], in_=h_ext[b0:b1])

            # constants: sel(p,c) keeps in_ where base + c - p == 0
            nc.vector.memset(ONES[:], 1.0)

            def sel(out_ap, in_ap, base, width=P0, fill=0.0):
                nc.gpsimd.affine_select(
                    out=out_ap, in_=in_ap, pattern=[[1, width]],
                    compare_op=mybir.AluOpType.is_equal, fill=fill,
                    base=base, channel_multiplier=-1,
                )

            # S1[p,c] = 1{c == p+1 and c % 14 != 0}: shift-by-one with
            # holes at n == 0 (those rows have no predecessor frame)
            sel(S16[:], ONES[:], -1)
            nc.gpsimd.affine_select(
                out=S16.rearrange("p (g m) -> p g m", g=P0 // 14 + 1, m=14)
                if False else S16[:, 0:112].rearrange("p (g m) -> p g m", g=8, m=14),
                in_=S16[:, 0:112].rearrange("p (g m) -> p g m", g=8, m=14),
                pattern=[[0, 8], [1, 14]],
                compare_op=mybir.AluOpType.is_gt, fill=0.0,
                base=0, channel_multiplier=0,
            )
            sel(EY[:], ONES[:], 0)           # EY[p,c]  = 1{c == p}
            # diag weights DG_j = diag(h[:, j]) in bf16
            DGS = {}
            for k in range(2):
                for j in PE_TAPS[k]:
                    DG = pool.tile(
                        [PN[k], PN[k]], BF16, tag=f"dg{k}_{j}", name=f"dg{k}_{j}"
                    )
                    DGS[(k, j)] = DG
            # tile0 diags on Pool via affine_select; tile1 diags on ACT via
            # EY*h so production runs on two engines in parallel
            for j in PE_TAPS[0]:
                sel(
                    DGS[(0, j)][:],
                    H[0][:, j : j + 1].broadcast_to([PR, PR]),
                    0,
                    width=PR,
                )

            # ACT: piece-A casts first so PE can start, then tile1 diags
            for k in range(2):
                nc.vector.memset(XA[k][:, 0:23], 0.0)
                nc.vector.memset(XC[k][:, 95:118], 0.0)
            for k in range(2):
                nc.scalar.copy(XA[k][:, 23:535], X[k][:, 0:512])
            for j in PE_TAPS[1]:
                nc.scalar.mul(DGS[(1, j)][:], EY[0:PR, 0:PR], H[1][:, j : j + 1])
            for k in range(2):
                nc.scalar.copy(XBt[k][:, 0:535], X[k][:, 489:1024])
                nc.scalar.copy(XC[k][:, 0:95], X[k][:, 1001:T])

            # DVE: tap 0 initializes Y[:, 0:T]; tail columns zeroed
            for k in range(2):
                nc.vector.memset(Y[k][:, T:F], 0.0)
                nc.vector.tensor_scalar_mul(Y[k][:, 0:T], X[k][:], H[k][:, 0:1])

            # PE: piece-major bf16 diag matmuls; per-piece groups retire
            # early so the DVE folds chase the PE instead of trailing it
            XP = [XA, XBt, XC]
            for p, (c0, c1) in enumerate(PIECES):
                w = c1 - c0
                for k in range(2):
                    taps = PE_TAPS[k]
                    for i, j in enumerate(taps):
                        nc.tensor.matmul(
                            PP[p][k][:, 0:w],
                            DGS[(k, j)][:],
                            XP[p][k][:, 23 - j : 23 - j + w],
                            start=(i == 0),
                            stop=(i == len(taps) - 1),
                        )

            # DVE: fp32 MAC taps into Y
            for k in range(2):
                for j in DVE_TAPS[k]:
                    nc.vector.scalar_tensor_tensor(
                        out=Y[k][:, j : j + T],
                        in0=X[k][:],
                        scalar=H[k][:, j : j + 1],
                        in1=Y[k][:, j : j + T],
                        op0=mybir.AluOpType.mult,
                        op1=mybir.AluOpType.add,
                    )

            # fold piece C first: the frame tails live in [1024, F), and the
            # tail-shift matmuls only need those columns
            for k in range(2):
                nc.vector.tensor_add(Y[k][:, 1024:F], Y[k][:, 1024:F], PC[k][:])

            # overlap-add: shift tails down one partition via the holed
            # shift matrix; the tile split is at a batch boundary so no
            # tail crosses tiles.
            for k in range(2):
                nc.tensor.matmul(
                    TP[k][:], S16[0:PR, 0:PR], Y[k][:, T:F],
                    start=True, stop=True,
                )

            # remaining folds, then the head adds
            for k in range(2):
                nc.vector.tensor_add(Y[k][:, 0:512], Y[k][:, 0:512], PA[k][:])
                nc.vector.tensor_add(Y[k][:, 512:1024], Y[k][:, 512:1024], PB[k][:])
            nc.vector.tensor_add(Y0[:, 0 : L - 1], Y0[:, 0 : L - 1], TP0[:])
            nc.vector.tensor_add(Y1[:, 0 : L - 1], Y1[:, 0 : L - 1], TP1[:])

            # stores: tile0 frames on SP, tile1 frames + last tails on ACT;
            # the last-frame tails are the n == 13 rows of each tile
            nc.sync.dma_start(out=ov[0:8], in_=Y0[:, 0:T])
            nc.scalar.dma_start(out=ov[8:BC], in_=Y1[:, 0:T])
            for k, (b0, b1) in enumerate(NS):
                nc.sync.dma_start(
                    out=out_ext[b0:b1, NB * T : OUT_LEN],
                    in_=Y[k].rearrange("(b n) f -> b n f", b=8, n=NB)[:, NB - 1, T:F],
                )
    _audit_single_wait(nc)
    return nc


def _get_nc():
    if "nc" not in _CACHE:
        _CACHE["nc"] = _build_nc()
    return _CACHE["nc"]


def _get_compiled():
    """Build the sharded PJRT callable once; reuse across kernel() calls."""
    if "jit" in _CACHE:
        return _CACHE["jit"]
    import jax
    from jax.sharding import Mesh, PartitionSpec
    from jax.experimental.shard_map import shard_map
    from concourse.bass2jax import (
        _bass_exec_p,
        install_neuronx_cc_hook,
        partition_id_tensor,
    )

    nc = _get_nc()
    install_neuronx_cc_hook()
    partition_name = nc.partition_id_tensor.name if nc.partition_id_tensor else None
    in_names, out_names, out_avals, zero_shapes = [], [], [], []
    for alloc in nc.m.functions[0].allocations:
        if not isinstance(alloc, mybir.MemoryLocationSet):
            continue
        name = alloc.memorylocations[0].name
        if alloc.kind == "ExternalInput":
            if name != partition_name:
                in_names.append(name)
        elif alloc.kind == "ExternalOutput":
            out_names.append(name)
            shape = tuple(alloc.tensor_shape)
            dtype = mybir.dt.np(alloc.dtype)
            out_avals.append(jax.core.ShapedArray(shape, dtype))
            zero_shapes.append((shape, dtype))
    n_params = len(in_names)
    all_in_names = list(in_names) + list(out_names)
    if partition_name is not None:
        all_in_names.append(partition_name)

    def _body(*args):
        operands = list(args)
        if partition_name is not None:
            operands.append(partition_id_tensor())
        outs = _bass_exec_p.bind(
            *operands,
            out_avals=tuple(out_avals),
            in_names=tuple(all_in_names),
            out_names=tuple(out_names),
            lowering_input_output_aliases=(),
            sim_require_finite=True,
            sim_require_nnan=True,
            nc=nc,
        )
        return tuple(outs)

    devices = jax.devices()[:N_CORES]
    mesh = Mesh(np.asarray(devices), ("core",))
    n_outs = len(out_names)
    in_specs = (PartitionSpec("core"),) * (n_params + n_outs)
    out_specs = (PartitionSpec("core"),) * n_outs
    f = jax.jit(
        shard_map(
            _body, mesh=mesh, in_specs=in_specs, out_specs=out_specs,
            check_rep=False,
        ),
        keep_unused=True,
    )
    # the kernel writes every output element, so the (non-donated) zero
    # buffers are placed on device once and reused
    zeros = [
        jax.device_put(np.zeros((N_CORES * s[0], *s[1:]), d))
        for (s, d) in zero_shapes
    ]
    _CACHE["jit"] = (f, in_names, zeros)
    return _CACHE["jit"]


def _run_traced(x, h, trace):
    """BASS_TRACE path: route through run_bass_kernel_spmd so external
    NTFF profiling hooks (where present) observe the execution."""
    from concourse.bass_utils import run_bass_kernel_spmd

    nc = _get_nc()
    in_maps = [
        {"x": x[i * BC : (i + 1) * BC], "h": h[i * BC : (i + 1) * BC]}
        for i in range(N_CORES)
    ]
    try:
        res = run_bass_kernel_spmd(nc, in_maps, list(range(N_CORES)), trace=trace)
    except ModuleNotFoundError:
        # no NTFF hook module in this environment — run untraced
        # (run_bass_kernel_spmd re-reads BASS_TRACE, so override it)
        os.environ["BASS_NEVER_TRACE"] = "1"
        try:
            res = run_bass_kernel_spmd(
                nc, in_maps, list(range(N_CORES)), trace=False
            )
        finally:
            del os.environ["BASS_NEVER_TRACE"]
    out = np.concatenate([res.results[i]["out"] for i in range(N_CORES)], axis=0)
    return out.astype(np.float32), res


def kernel(x, h_time):
    x = np.ascontiguousarray(np.asarray(x, dtype=np.float32))
    h = np.ascontiguousarray(np.asarray(h_time, dtype=np.float32))
    if os.environ.get("BASS_TRACE"):
        out, _ = _run_traced(x, h, True)
        return out
    import jax

    f, in_names, zeros = _get_compiled()
    arrs = {"x": x, "h": h}
    dev_in = [jax.device_put(arrs[name]) for name in in_names]
    outs = f(*dev_in, *zeros)
    return np.asarray(outs[0]).astype(np.float32)


if __name__ == "__main__":
    # Dry build: construct the program and report instruction counts.
    nc = _build_nc()
    from collections import Counter

    cnt = Counter()
    for f in nc.m.functions:
        for blk in f.blocks:
            for ins in blk.instructions:
                cnt[type(ins).__name__] += 1
    print(dict(cnt))
    print("total instructions:", sum(cnt.values()))


# revision 9
# speedup vs baseline: 1.0709x; 1.0709x over previous
"""Trainium2 Bass kernel for CustomApplyTimeChannel.

Per (batch b, block n): y[b,n,:] = full_conv(x[b,n,:1096], h[b,n,:24]),
then overlap-add with hop T=1096 into out[b, :15367].
Pure data parallel over batch across 8 NeuronCores (16 b/core); per-core
rows p = b*14 + n (b-major), split into two 112-row partition tiles at a
batch boundary.  b-major descriptor order walks DRAM contiguously, which
measures ~10x faster per body on real HW than the n-major order (the DMA
descriptor path, not compute, dominates real execution time here).

Engine split of the 24 taps (per tile):
  - taps 1..18 / 1..17 on PE as bf16 diag-weight matmuls accumulating
    fp32 in PSUM.  The ISA caps a matmul at one PSUM bank (512 fp32
    cols), so the 1119 output columns are split into three pieces with
    SEPARATE PSUM tiles and SEPARATE bf16 x-window tiles per piece —
    Tile's dependency tracking is tile-coarse, so per-piece tiles let
    each piece's fold run as soon as that piece's group retires.
  - tap 0 + the remaining taps on DVE as fp32 TensorScalarPtr MACs.
  - Constants are built on-device: tile-0 diag weights + the two
    overlap-add shift matrices on Pool via affine_select (from a
    stride-0 broadcast of h), tile-1 diag weights on ACT as eye*h.
  - ACT casts x to bf16 (three window tiles per partition tile).
The overlap-add shifts frame tails down one partition with a holed
shift-matrix matmul per tile into PSUM plus two narrow DVE adds (the
tile split is at a batch boundary, so no tail crosses tiles).  All DMAs ride
the SP/ACT hardware DGE queues (gpsimd software DGE is far slower per
descriptor).

This walrus accepts only ONE sync wait per instruction; any multi-wait
instruction is legalized post-schedule by hoisting extra waits onto
single-wait Drain instructions on the same engine (see
_legalize_single_wait).

The host path compiles the PJRT executable once and caches it; repeat
kernel() calls only device_put the inputs and dispatch.  If BASS_TRACE
is set, execution routes through run_bass_kernel_spmd so NTFF profiling
hooks (where available) still observe the kernel.
"""

import os
import sys

sys.path.insert(0, "/opt/trn_rl_repo")

import numpy as np

from concourse import bass, tile, mybir

# Problem constants (hardcoded; kernel.py must be self-contained).
B = 128          # total batch
NB = 14          # channel blocks
T = 1096         # time samples per block
L = 24           # taps
F = T + L - 1    # frame length 1119
OUT_LEN = (NB - 1) * T + F   # 15367
N_CORES = 8
BC = B // N_CORES            # 16 batches per core
ROWS = NB * BC               # 224 rows per core
P0 = 128                     # constant-tile partition width
PR = 112                     # rows per partition tile (8 batches x 14 blocks)

FP32 = mybir.dt.float32
BF16 = mybir.dt.bfloat16

# per-tile tap assignment (tap 0 initializes Y on DVE)
PE_TAPS = [list(range(1, 19)), list(range(1, 18))]
DVE_TAPS = [list(range(19, 24)), list(range(18, 24))]

# rows are b-major (p = b*14 + n): DMA descriptors then walk DRAM
# contiguously, which measures ~2x faster per body on real HW than the
# n-major order; the two 112-row tiles split at a batch boundary, so no
# frame tail crosses tiles (n = 0 rows start each batch).

# column pieces of the 1119-wide output, one PSUM bank each
PIECES = [(0, 512), (512, 1024), (1024, F)]

_CACHE = {}


def _legalize_single_wait(nc):
    """The walrus build here allows ONE sync wait per instruction.  For any
    instruction carrying N>1 waits, hoist N-1 of them onto bare Drain
    instructions on the SAME engine placed immediately before it: the engine
    sequencer executes in order, so the conjunction of waits is preserved.
    Must run after Tile's scheduler assigned sync_info and after the Tile
    instruction hook was popped (i.e. from within _drain_and_barrier)."""
    for f in nc.m.functions:
        for blk in f.blocks:
            snapshot = list(blk.instructions)
            if not any(
                i.sync_info is not None and len(i.sync_info.on_wait) > 1
                for i in snapshot
            ):
                continue
            created = set()
            new_list = []
            for ins in snapshot:
                si = ins.sync_info
                if si is not None and len(si.on_wait) > 1:
                    waits = list(si.on_wait)
                    for w in waits[:-1]:
                        d = nc.sync.drain()
                        d.ins.engine = ins.engine
                        d.ins.sync_info = mybir.SyncInfo(on_wait=[w], on_update=[])
                        created.add(id(d.ins))
                        new_list.append(d.ins)
                    ins.sync_info = mybir.SyncInfo(
                        on_wait=[waits[-1]], on_update=list(si.on_update)
                    )
                new_list.append(ins)
            # nc.sync.drain() appended the new drains at the end of the
            # current block; drop those trailing copies everywhere and
            # install the ordered list.
            for f2 in nc.m.functions:
                for blk2 in f2.blocks:
                    if blk2 is blk:
                        blk2.instructions[:] = new_list
                    else:
                        blk2.instructions[:] = [
                            i for i in blk2.instructions if id(i) not in created
                        ]


def _patch_drain_split():
    """Tile's kernel-tail drain carries one wait per outstanding processor;
    split it into a chain of single-wait drains, and legalize any other
    multi-wait instruction the same way."""
    if getattr(tile.TileContext, "_drain_split_patched", False):
        return
    from concourse.vector_clock import ScopedClock

    def _drain_and_barrier(self, tick_clock, wait_clock):
        _legalize_single_wait(self.nc)
        drain_inst = self.nc.sync.drain()
        wait_clock.add_sem_waits(
            drain_inst.ins, ScopedClock({None: tick_clock.global_clock})
        )
        si = drain_inst.ins.sync_info
        if si is not None and len(si.on_wait) > 1:
            waits = list(si.on_wait)
            drain_inst.ins.sync_info = mybir.SyncInfo(
                on_wait=[waits[0]], on_update=list(si.on_update)
            )
            for w in waits[1:]:
                d2 = self.nc.sync.drain()
                d2.ins.sync_info = mybir.SyncInfo(on_wait=[w], on_update=[])
        self.nc.all_engine_barrier()
        popped = self.nc._tile_sem_poison_stack.pop()
        assert popped is self._sem_poison
        self.nc.clear_and_free_semaphores(list(self.sems.allocated().values()))
        self.nc.all_engine_barrier()

    tile.TileContext._drain_and_barrier = _drain_and_barrier
    tile.TileContext._drain_split_patched = True


def _audit_single_wait(nc):
    bad = []
    for f in nc.m.functions:
        for blk in f.blocks:
            for ins in blk.instructions:
                si = ins.sync_info
                if si is not None and len(si.on_wait) > 1:
                    bad.append((type(ins).__name__, ins.name, len(si.on_wait)))
    if bad:
        raise RuntimeError(f"instructions with >1 sync wait: {bad}")


def _build_nc():
    _patch_drain_split()
    nc = bass.Bass()
    x_ext = nc.declare_dram_parameter("x", [BC, NB, T], FP32, isOutput=False)
    h_ext = nc.declare_dram_parameter("h", [BC, NB, L], FP32, isOutput=False)
    out_ext = nc.declare_dram_parameter("out", [BC, OUT_LEN], FP32, isOutput=True)

    ov = out_ext[:, : NB * T].rearrange("b (n t) -> b n t", n=NB, t=T)

    with tile.TileContext(nc) as tc:
        with (
            tc.tile_pool(name="main", bufs=1) as pool,
            tc.tile_pool(name="ps", bufs=1, space="PSUM") as pspool,
        ):
            ONES = pool.tile([P0, P0], FP32, tag="ones")
            S16 = pool.tile([P0, P0], FP32, tag="s16")
            EY = pool.tile([P0, P0], FP32, tag="ey")
            X0 = pool.tile([PR, T], FP32, tag="x0")
            X1 = pool.tile([PR, T], FP32, tag="x1")
            H0 = pool.tile([PR, L], FP32, tag="h0")
            H1 = pool.tile([PR, L], FP32, tag="h1")
            Y0 = pool.tile([PR, F], FP32, tag="y0")
            Y1 = pool.tile([PR, F], FP32, tag="y1")
            X, H, Y, PN = [X0, X1], [H0, H1], [Y0, Y1], [PR, PR]
            # per-piece bf16 x windows: piece p of tap j reads columns
            # [23-j+c0, 23-j+c1) of the padded x; with per-piece tiles the
            # local window is [23-j, 23-j+piece_w) in every piece.
            XA = [
                pool.tile([PN[k], 535], BF16, tag=f"xa{k}", name=f"xa{k}")
                for k in range(2)
            ]
            XBt = [
                pool.tile([PN[k], 535], BF16, tag=f"xb{k}", name=f"xb{k}")
                for k in range(2)
            ]
            XC = [
                pool.tile([PN[k], 118], BF16, tag=f"xc{k}", name=f"xc{k}")
                for k in range(2)
            ]
            # per-piece PSUM accumulators + tail-shift tiles: 8 banks exactly
            PA = [
                pspool.tile([PN[k], 512], FP32, tag=f"pa{k}", name=f"pa{k}")
                for k in range(2)
            ]
            PB = [
                pspool.tile([PN[k], 512], FP32, tag=f"pb{k}", name=f"pb{k}")
                for k in range(2)
            ]
            PC = [
                pspool.tile([PN[k], F - 1024], FP32, tag=f"pc{k}", name=f"pc{k}")
                for k in range(2)
            ]
            PP = [PA, PB, PC]
            TP0 = pspool.tile([PR, L - 1], FP32, tag="tp0")
            TP1 = pspool.tile([PR, L - 1], FP32, tag="tp1")
            TP = [TP0, TP1]

            # loads: x on SP, h on ACT (both hardware DGE), b-major
            NS = [(0, 8), (8, BC)]
            for k, (b0, b1) in enumerate(NS):
                nc.sync.dma_start(out=X[k][:], in_=x_ext[b0:b1])
                nc.scalar.dma_start(out=H[k][:], in_=h_ext[b0:b1])

            # constants: sel(p,c) keeps in_ where base + c - p == 0
            nc.vector.memset(ONES[:], 1.0)

            def sel(out_ap, in_ap, base, width=P0, fill=0.0):
                nc.gpsimd.affine_select(
                    out=out_ap, in_=in_ap, pattern=[[1, width]],
                    compare_op=mybir.AluOpType.is_equal, fill=fill,
                    base=base, channel_multiplier=-1,
                )

            # S1[p,c] = 1{c == p+1 and c % 14 != 0}: shift-by-one with
            # holes at n == 0 (those rows have no predecessor frame)
            sel(S16[:], ONES[:], -1)
            nc.gpsimd.affine_select(
                out=S16.rearrange("p (g m) -> p g m", g=P0 // 14 + 1, m=14)
                if False else S16[:, 0:112].rearrange("p (g m) -> p g m", g=8, m=14),
                in_=S16[:, 0:112].rearrange("p (g m) -> p g m", g=8, m=14),
                pattern=[[0, 8], [1, 14]],
                compare_op=mybir.AluOpType.is_gt, fill=0.0,
                base=0, channel_multiplier=0,
            )
            sel(EY[:], ONES[:], 0)           # EY[p,c]  = 1{c == p}
            # diag weights DG_j = diag(h[:, j]) in bf16
            DGS = {}
            for k in range(2):
                for j in PE_TAPS[k]:
                    DG = pool.tile(
                        [PN[k], PN[k]], BF16, tag=f"dg{k}_{j}", name=f"dg{k}_{j}"
                    )
                    DGS[(k, j)] = DG
            # tile0 diags on Pool via affine_select; tile1 diags on ACT via
            # EY*h so production runs on two engines in parallel
            for j in PE_TAPS[0]:
                sel(
                    DGS[(0, j)][:],
                    H[0][:, j : j + 1].broadcast_to([PR, PR]),
                    0,
                    width=PR,
                )

            # ACT: piece-A casts first so PE can start, then tile1 diags
            for k in range(2):
                nc.vector.memset(XA[k][:, 0:23], 0.0)
                nc.vector.memset(XC[k][:, 95:118], 0.0)
            for k in range(2):
                nc.scalar.copy(XA[k][:, 23:535], X[k][:, 0:512])
            for j in PE_TAPS[1]:
                nc.scalar.mul(DGS[(1, j)][:], EY[0:PR, 0:PR], H[1][:, j : j + 1])
            for k in range(2):
                nc.scalar.copy(XBt[k][:, 0:535], X[k][:, 489:1024])
                nc.scalar.copy(XC[k][:, 0:95], X[k][:, 1001:T])

            # DVE: tap 0 initializes Y[:, 0:T]; tail columns zeroed
            for k in range(2):
                nc.vector.memset(Y[k][:, T:F], 0.0)
                nc.vector.tensor_scalar_mul(Y[k][:, 0:T], X[k][:], H[k][:, 0:1])

            # PE: piece-major bf16 diag matmuls; per-piece groups retire
            # early so the DVE folds chase the PE instead of trailing it.
            # Order A, C, B: the tiny C piece (frame tails) retires
            # mid-stream, unblocking the tail-shift machinery before the
            # last big piece finishes.
            XP = [XA, XBt, XC]
            for p, (c0, c1) in sorted(
                enumerate(PIECES), key=lambda e: (0, 2, 1)[e[0]]
            ):
                w = c1 - c0
                for k in range(2):
                    taps = PE_TAPS[k]
                    for i, j in enumerate(taps):
                        nc.tensor.matmul(
                            PP[p][k][:, 0:w],
                            DGS[(k, j)][:],
                            XP[p][k][:, 23 - j : 23 - j + w],
                            start=(i == 0),
                            stop=(i == len(taps) - 1),
                        )

            # DVE: fp32 MAC taps into Y
            for k in range(2):
                for j in DVE_TAPS[k]:
                    nc.vector.scalar_tensor_tensor(
                        out=Y[k][:, j : j + T],
                        in0=X[k][:],
                        scalar=H[k][:, j : j + 1],
                        in1=Y[k][:, j : j + T],
                        op0=mybir.AluOpType.mult,
                        op1=mybir.AluOpType.add,
                    )

            # fold piece C first: the frame tails live in [1024, F), and the
            # tail-shift matmuls only need those columns
            for k in range(2):
                nc.vector.tensor_add(Y[k][:, 1024:F], Y[k][:, 1024:F], PC[k][:])

            # overlap-add: shift tails down one partition via the holed
            # shift matrix; the tile split is at a batch boundary so no
            # tail crosses tiles.
            for k in range(2):
                nc.tensor.matmul(
                    TP[k][:], S16[0:PR, 0:PR], Y[k][:, T:F],
                    start=True, stop=True,
                )

            # remaining folds, then the head adds
            for k in range(2):
                nc.vector.tensor_add(Y[k][:, 0:512], Y[k][:, 0:512], PA[k][:])
                nc.vector.tensor_add(Y[k][:, 512:1024], Y[k][:, 512:1024], PB[k][:])
            nc.vector.tensor_add(Y0[:, 0 : L - 1], Y0[:, 0 : L - 1], TP0[:])
            nc.vector.tensor_add(Y1[:, 0 : L - 1], Y1[:, 0 : L - 1], TP1[:])

            # stores: tile0 frames on SP, tile1 frames + last tails on ACT;
            # the last-frame tails are the n == 13 rows of each tile
            nc.sync.dma_start(out=ov[0:8], in_=Y0[:, 0:T])
            nc.scalar.dma_start(out=ov[8:BC], in_=Y1[:, 0:T])
            for k, (b0, b1) in enumerate(NS):
                nc.sync.dma_start(
                    out=out_ext[b0:b1, NB * T : OUT_LEN],
                    in_=Y[k].rearrange("(b n) f -> b n f", b=8, n=NB)[:, NB - 1, T:F],
                )
    _audit_single_wait(nc)
    return nc


def _get_nc():
    if "nc" not in _CACHE:
        _CACHE["nc"] = _build_nc()
    return _CACHE["nc"]


def _get_compiled():
    """Build the sharded PJRT callable once; reuse across kernel() calls."""
    if "jit" in _CACHE:
        return _CACHE["jit"]
    import jax
    from jax.sharding import Mesh, PartitionSpec
    from jax.experimental.shard_map import shard_map
    from concourse.bass2jax import (
        _bass_exec_p,
        install_neuronx_cc_hook,
        partition_id_tensor,
    )

    nc = _get_nc()
    install_neuronx_cc_hook()
    partition_name = nc.partition_id_tensor.name if nc.partition_id_tensor else None
    in_names, out_names, out_avals, zero_shapes = [], [], [], []
    for alloc in nc.m.functions[0].allocations:
        if not isinstance(alloc, mybir.MemoryLocationSet):
            continue
        name = alloc.memorylocations[0].name
        if alloc.kind == "ExternalInput":
            if name != partition_name:
                in_names.append(name)
        elif alloc.kind == "ExternalOutput":
            out_names.append(name)
            shape = tuple(alloc.tensor_shape)
            dtype = mybir.dt.np(alloc.dtype)
            out_avals.append(jax.core.ShapedArray(shape, dtype))
            zero_shapes.append((shape, dtype))
    n_params = len(in_names)
    all_in_names = list(in_names) + list(out_names)
    if partition_name is not None:
        all_in_names.append(partition_name)

    def _body(*args):
        operands = list(args)
        if partition_name is not None:
            operands.append(partition_id_tensor())
        outs = _bass_exec_p.bind(
            *operands,
            out_avals=tuple(out_avals),
            in_names=tuple(all_in_names),
            out_names=tuple(out_names),
            lowering_input_output_aliases=(),
            sim_require_finite=True,
            sim_require_nnan=True,
            nc=nc,
        )
        return tuple(outs)

    devices = jax.devices()[:N_CORES]
    mesh = Mesh(np.asarray(devices), ("core",))
    n_outs = len(out_names)
    in_specs = (PartitionSpec("core"),) * (n_params + n_outs)
    out_specs = (PartitionSpec("core"),) * n_outs
    f = jax.jit(
        shard_map(
            _body, mesh=mesh, in_specs=in_specs, out_specs=out_specs,
            check_rep=False,
        ),
        keep_unused=True,
    )
    # the kernel writes every output element, so the (non-donated) zero
    # buffers are placed on device once and reused
    zeros = [
        jax.device_put(np.zeros((N_CORES * s[0], *s[1:]), d))
        for (s, d) in zero_shapes
    ]
    _CACHE["jit"] = (f, in_names, zeros)
    return _CACHE["jit"]


def _run_traced(x, h, trace):
    """BASS_TRACE path: route through run_bass_kernel_spmd so external
    NTFF profiling hooks (where present) observe the execution."""
    from concourse.bass_utils import run_bass_kernel_spmd

    nc = _get_nc()
    in_maps = [
        {"x": x[i * BC : (i + 1) * BC], "h": h[i * BC : (i + 1) * BC]}
        for i in range(N_CORES)
    ]
    try:
        res = run_bass_kernel_spmd(nc, in_maps, list(range(N_CORES)), trace=trace)
    except ModuleNotFoundError:
        # no NTFF hook module in this environment — run untraced
        # (run_bass_kernel_spmd re-reads BASS_TRACE, so override it)
        os.environ["BASS_NEVER_TRACE"] = "1"
        try:
            res = run_bass_kernel_spmd(
                nc, in_maps, list(range(N_CORES)), trace=False
            )
        finally:
            del os.environ["BASS_NEVER_TRACE"]
    out = np.concatenate([res.results[i]["out"] for i in range(N_CORES)], axis=0)
    return out.astype(np.float32), res


def kernel(x, h_time):
    x = np.ascontiguousarray(np.asarray(x, dtype=np.float32))
    h = np.ascontiguousarray(np.asarray(h_time, dtype=np.float32))
    if os.environ.get("BASS_TRACE"):
        out, _ = _run_traced(x, h, True)
        return out
    import jax

    f, in_names, zeros = _get_compiled()
    arrs = {"x": x, "h": h}
    dev_in = [jax.device_put(arrs[name]) for name in in_names]
    outs = f(*dev_in, *zeros)
    return np.asarray(outs[0]).astype(np.float32)


if __name__ == "__main__":
    # Dry build: construct the program and report instruction counts.
    nc = _build_nc()
    from collections import Counter

    cnt = Counter()
    for f in nc.m.functions:
        for blk in f.blocks:
            for ins in blk.instructions:
                cnt[type(ins).__name__] += 1
    print(dict(cnt))
    print("total instructions:", sum(cnt.values()))
